# revision 7
# baseline (speedup 1.0000x reference)
import sys

sys.path.insert(0, "/opt/trn_rl_repo")
import numpy as np

B, DIM, H, W = 2, 192, 64, 64
HEADS = 4
C = DIM // HEADS  # 48 per-head channels
HW = H * W  # 4096
NCORES = 8
QQ = 4  # q quarters of 1024
QW = HW // QQ  # 1024
KCH = HW // 128  # 32 k-chunks

_cache = {}


def _emit_av(nc, avP, vT, rec, npairs):
    ka, Ea, kb, Eb, first = rec
    last = kb == 2 * npairs - 1
    for nn in range(2):
        ns = slice(512 * nn, 512 * (nn + 1))
        nc.tensor.matmul(avP[:, ns], vT[:, 49 * ka:49 * ka + 49],
                         Ea[:, ns], start=first, stop=False,
                         skip_group_check=True)
        nc.tensor.matmul(avP[:, ns], vT[:, 49 * kb:49 * kb + 49],
                         Eb[:, ns], start=False, stop=last,
                         skip_group_check=True)


def _build():
    import concourse.bass as bass
    import concourse.tile as tile
    from concourse import bacc, mybir

    F32 = mybir.dt.float32
    BF16 = mybir.dt.float16
    AF = mybir.ActivationFunctionType

    nc = bacc.Bacc("TRN2", target_bir_lowering=False, debug=False,
                   num_devices=NCORES)
    x_d = nc.dram_tensor("x", [DIM, HW], BF16, kind="ExternalInput").ap()
    w1a_d = nc.dram_tensor("w1a", [128, 3 * C], BF16, kind="ExternalInput").ap()
    w1b_d = nc.dram_tensor("w1b", [64, 3 * C], BF16, kind="ExternalInput").ap()
    dwqk_d = nc.dram_tensor("dwqk", [128, 9 * C], BF16, kind="ExternalInput").ap()
    dwv_d = nc.dram_tensor("dwv", [C, 9 * C], BF16, kind="ExternalInput").ap()
    pw_d = nc.dram_tensor("pw", [128, 128], BF16, kind="ExternalInput").ap()
    id_d = nc.dram_tensor("ident", [128, 128], F32, kind="ExternalInput").ap()
    tp_d = nc.dram_tensor("temp", [1, 1], F32, kind="ExternalInput").ap()
    out_d = nc.dram_tensor("out", [DIM, HW], F32, kind="ExternalOutput").ap()

    with tile.TileContext(nc) as tc:
        with (
            tc.tile_pool(name="persist", bufs=1) as pp,
            tc.tile_pool(name="epool", bufs=8) as ep,
        ):
            # ---- persistent SBUF tiles
            PK = pp.tile([128, HW], F32, tag="PK")     # q'@0:48, k'@64:112
            QHD = pp.tile([128, HW], BF16, tag="QHD")  # qhat bf16 dup'd 0:48 & 64:112
            KHb = pp.tile([128, HW], BF16, tag="KHb")  # k' bf16 at 0:48 and 64:112
            vT = pp.tile([128, KCH * (C + 1)], BF16, tag="vT")  # v^T bf16 + ones col
            U = pp.tile([49, HW], F32, tag="U")        # av accum (row 48 = Z)
            ident = pp.tile([128, 128], F32, tag="ident")
            w1a = pp.tile([128, 3 * C], BF16, tag="w1a")
            w1b = pp.tile([64, 3 * C], BF16, tag="w1b")
            dwqk = pp.tile([128, 9 * C], BF16, tag="dwqk")
            dwv = pp.tile([C, 9 * C], BF16, tag="dwv")
            pw = pp.tile([128, 128], BF16, tag="pw")
            temp_sb = pp.tile([1, 1], F32, tag="temp")
            ones_row = pp.tile([1, 128], F32, tag="ones_row")
            ones48 = pp.tile([128, 1], F32, tag="ones48")
            temp_col = pp.tile([128, 1], F32, tag="temp_col")
            negtemp_col = pp.tile([128, 1], F32, tag="negtemp_col")
            rr2 = pp.tile([128, 64], F32, tag="rr2")   # rq | rk (1/||.||)
            rkt = pp.tile([128, KCH], F32, tag="rkt")  # temp * rk, [p, chunk]
            att = pp.tile([128, HW], BF16, tag="att")
            z_row = pp.tile([1, HW], F32, tag="z_row")
            rz_row = pp.tile([1, HW], F32, tag="rz_row")
            rz = pp.tile([128, KCH], F32, tag="rz")

            nc.sync.dma_start(ident[:], id_d[:])
            nc.sync.dma_start(w1a[:], w1a_d[:])
            nc.sync.dma_start(w1b[:], w1b_d[:])
            nc.sync.dma_start(dwqk[:], dwqk_d[:])
            nc.sync.dma_start(dwv[:], dwv_d[:])
            nc.sync.dma_start(pw[:], pw_d[:])
            nc.sync.dma_start(temp_sb[:], tp_d[:])
            nc.gpsimd.memset(ones_row[:], 1.0)
            nc.gpsimd.memset(ones48[:], 1.0)
            nc.gpsimd.memset(vT[:], 1.0)

            with (
                tc.tile_pool(name="ph12", bufs=1) as p12,
                tc.tile_pool(name="psA", bufs=4, space="PSUM") as psA,
                tc.tile_pool(name="psB", bufs=2, space="PSUM") as psB,
            ):
                TQK = p12.tile([128, HW], BF16, tag="TQK")
                TV = p12.tile([C, HW], BF16, tag="TV")
                v_sb = p12.tile([C, HW], F32, tag="v_sb")

                with tc.tile_pool(name="xp", bufs=1) as xp:
                    x_a = xp.tile([128, HW], BF16, tag="x_a")
                    x_b = xp.tile([64, HW], BF16, tag="x_b")
                    nc.sync.dma_start(x_a[:], x_d[0:128, :])
                    nc.sync.dma_start(x_b[:], x_d[128:192, :])

                    # temp_col = broadcast temp over partitions (K=1 matmul)
                    tP = psB.tile([128, 1], F32, tag="b")
                    nc.tensor.matmul(tP[:], ones_row[0:1, :], temp_sb[:],
                                     start=True, stop=True)
                    nc.scalar.copy(temp_col[:], tP[:])
                    nc.scalar.activation(negtemp_col[:], tP[:], AF.Copy,
                                         scale=-1.0)

                    # ---- phase 1: 1x1 conv (qkv_w), q/k col-packed, v separate
                    for n in range(8):
                        s = slice(512 * n, 512 * (n + 1))
                        T1 = psA.tile([128, 512], F32, tag="a")
                        nc.tensor.matmul(T1[0:48, :], w1a[:, 0:48], x_a[:, s],
                                         start=True, stop=False)
                        nc.tensor.matmul(T1[64:112, :], w1a[:, 48:96], x_a[:, s],
                                         start=True, stop=False)
                        nc.tensor.matmul(T1[0:48, :], w1b[:, 0:48], x_b[:, s],
                                         start=False, stop=True)
                        nc.tensor.matmul(T1[64:112, :], w1b[:, 48:96], x_b[:, s],
                                         start=False, stop=True)
                        T1v = psB.tile([48, 512], F32, tag="b")
                        nc.tensor.matmul(T1v[:], w1a[:, 96:144], x_a[:, s],
                                         start=True, stop=False)
                        nc.tensor.matmul(T1v[:], w1b[:, 96:144], x_b[:, s],
                                         start=False, stop=True)
                        eng = nc.vector if n % 2 == 0 else nc.scalar
                        if eng is nc.vector:
                            nc.vector.tensor_copy(TQK[:, s], T1[:])
                            nc.scalar.copy(TV[:, s], T1v[:])
                        else:
                            nc.scalar.copy(TQK[:, s], T1[:])
                            nc.vector.tensor_copy(TV[:, s], T1v[:])

                ph35 = tc.tile_pool(name="ph35", bufs=1)
                p35 = ph35.__enter__()
                SQ = p35.tile([112, HW], F32, tag="SQ")
                ss_sb = p35.tile([33, HW], F32, tag="ss_sb")
                rq_row = p35.tile([1, HW], F32, tag="rq_row")

                # ---- phase 2: depthwise 3x3 (diag matmuls, 9 taps, crop APs)
                TQK3 = TQK[:].rearrange("p (y x) -> p y x", x=64)
                TV3 = TV[:].rearrange("p (y x) -> p y x", x=64)
                taps = [(0, 0)] + [(dy, dx) for dy in (-1, 0, 1)
                                   for dx in (-1, 0, 1) if (dy, dx) != (0, 0)]
                for n in range(8):
                    s = slice(512 * n, 512 * (n + 1))
                    DQK = psA.tile([128, 8, 64], F32, tag="a")
                    DV = psB.tile([48, 8, 64], F32, tag="b")
                    for ti, (dy, dx) in enumerate(taps):
                        t = (dy + 1) * 3 + (dx + 1)
                        first = ti == 0
                        last = ti == len(taps) - 1
                        gy0, gy1 = max(0, -dy), 64 - max(0, dy)
                        sy0, sy1 = max(8 * n, gy0), min(8 * n + 8, gy1)
                        if sy1 <= sy0:
                            continue
                        x0, x1 = max(0, -dx), 64 - max(0, dx)
                        oy = slice(sy0 - 8 * n, sy1 - 8 * n)
                        ox = slice(x0, x1)
                        iy = slice(sy0 + dy, sy1 + dy)
                        ix = slice(x0 + dx, x1 + dx)
                        wsl = slice(C * t, C * t + 48)
                        nc.tensor.matmul(
                            DQK[0:48, oy, ox], dwqk[0:48, wsl],
                            TQK3[0:48, iy, ix], start=first, stop=last,
                            skip_group_check=True)
                        nc.tensor.matmul(
                            DQK[64:112, oy, ox], dwqk[64:112, wsl],
                            TQK3[64:112, iy, ix], start=first, stop=last,
                            skip_group_check=True)
                        nc.tensor.matmul(
                            DV[:, oy, ox], dwv[:, wsl],
                            TV3[:, iy, ix], start=first, stop=last,
                            skip_group_check=True)
                    DQKf = DQK[:].rearrange("p y x -> p (y x)")
                    DVf = DV[:].rearrange("p y x -> p (y x)")
                    if n % 2 == 0:
                        nc.vector.tensor_copy(PK[:, s], DQKf[:])
                        nc.scalar.copy(v_sb[:, s], DVf[:])
                    else:
                        nc.scalar.copy(PK[:, s], DQKf[:])
                        nc.vector.tensor_copy(v_sb[:, s], DVf[:])
                    nc.vector.tensor_copy(KHb[64:112, s],
                                          DQK[64:112, :, :].rearrange(
                                              "p y x -> p (y x)"))

                # k' bf16 duplicate at base 0 (for even k-chunk lhsT)
                nc.sync.dma_start(KHb[0:48, :], KHb[64:112, :])

                # ---- phase 3: squares + sum-of-squares (per spatial position)
                for n in range(8):
                    s = slice(512 * n, 512 * (n + 1))
                    nc.vector.tensor_mul(SQ[:, s], PK[0:112, s], PK[0:112, s])
                    ssP = psA.tile([128, 512], F32, tag="a")
                    nc.tensor.matmul(ssP[0:1, :], ones48[0:48, :], SQ[0:48, s],
                                     start=True, stop=True)
                    nc.tensor.matmul(ssP[32:33, :], ones48[64:112, :],
                                     SQ[64:112, s], start=True, stop=True)
                    nc.vector.tensor_copy(ss_sb[:, s], ssP[0:33, :])

                # ---- phase 4: rsqrt via exp(-0.5*ln(ss)) in [128, 32] layout
                ssqT = psA.tile([128, 32], F32, tag="a")
                sskT = psA.tile([128, 32], F32, tag="a")
                for j in range(KCH):
                    cs = slice(128 * j, 128 * (j + 1))
                    nc.tensor.transpose(ssqT[:, j:j + 1], ss_sb[0:1, cs],
                                        ident[0:1, 0:1])
                    nc.tensor.transpose(sskT[:, j:j + 1], ss_sb[32:33, cs],
                                        ident[32:33, 32:33])
                lnb = pp.tile([128, 64], F32, tag="lnb")
                nc.scalar.activation(lnb[:, 0:32], ssqT[:], AF.Ln)
                nc.scalar.activation(lnb[:, 32:64], sskT[:], AF.Ln)
                nc.scalar.activation(rr2[:], lnb[:], AF.Exp, scale=-0.5)
                nc.scalar.activation(rkt[:], rr2[:, 32:64], AF.Copy,
                                     scale=temp_col[:])

                # ---- phase 5: rq -> row layout, broadcast, normalize q
                for g in range(8):
                    rqP = psA.tile([1, 512], F32, tag="a")
                    for jj in range(4):
                        j = 4 * g + jj
                        nc.tensor.transpose(rqP[0:1, 128 * jj:128 * (jj + 1)],
                                            rr2[:, j:j + 1], ident[:])
                    nc.scalar.copy(rq_row[0:1, 512 * g:512 * (g + 1)], rqP[:])
                for n in range(8):
                    s = slice(512 * n, 512 * (n + 1))
                    rqbP = psB.tile([48, 512], F32, tag="b")
                    nc.tensor.matmul(rqbP[:], ones_row[0:1, 0:48],
                                     rq_row[0:1, s], start=True, stop=True)
                    nc.vector.tensor_mul(QHD[0:48, s], PK[0:48, s], rqbP[:])
                nc.sync.dma_start(QHD[64:112, :], QHD[0:48, :])

                # ---- phase 6: v transpose -> vT chunks [128, 49] (ones col)
                for j in range(KCH):
                    vtP = psB.tile([128, 48], F32, tag="b")
                    nc.tensor.transpose(vtP[:], v_sb[:, 128 * j:128 * (j + 1)],
                                        ident[0:48, 0:48])
                    eng = nc.vector if j % 2 == 0 else nc.scalar
                    if j % 2 == 0:
                        nc.vector.tensor_copy(
                            vT[:, 49 * j:49 * j + 48], vtP[:])
                    else:
                        nc.scalar.copy(vT[:, 49 * j:49 * j + 48], vtP[:])
                ph35.__exit__(None, None, None)

            # ---- phase 7: attention (ACT-exp bound steady state)
            with (
                tc.tile_pool(name="psS", bufs=3, space="PSUM") as psS,
                tc.tile_pool(name="psAV", bufs=1, space="PSUM") as psAV,
            ):
                for qq in range(QQ):
                    q0 = QW * qq
                    avP = psAV.tile([49, QW], F32, tag="av")
                    pend = []
                    for i in range(KCH // 2):
                        ka, kb = 2 * i, 2 * i + 1
                        Sa = psS.tile([128, QW], F32, tag="S")
                        Sb = psS.tile([128, QW], F32, tag="S")
                        for nn in range(2):
                            qs = slice(q0 + 512 * nn, q0 + 512 * (nn + 1))
                            nc.tensor.matmul(
                                Sa[:, 512 * nn:512 * (nn + 1)],
                                KHb[0:48, 128 * ka:128 * (ka + 1)],
                                QHD[0:48, qs], start=True, stop=True)
                            nc.tensor.matmul(
                                Sb[:, 512 * nn:512 * (nn + 1)],
                                KHb[64:112, 128 * kb:128 * (kb + 1)],
                                QHD[64:112, qs], start=True, stop=True)
                        Ea = ep.tile([128, QW], BF16, tag="E")
                        Eb = ep.tile([128, QW], BF16, tag="E")
                        nc.scalar.activation(Ea[:], Sa[:], AF.Exp,
                                             bias=negtemp_col[:],
                                             scale=rkt[:, ka:ka + 1])
                        nc.scalar.activation(Eb[:], Sb[:], AF.Exp,
                                             bias=negtemp_col[:],
                                             scale=rkt[:, kb:kb + 1])
                        pend.append((ka, Ea, kb, Eb, i == 0))
                        if i > 0:
                            _emit_av(nc, avP, vT, pend.pop(0), KCH // 2)
                    while pend:
                        _emit_av(nc, avP, vT, pend.pop(0), KCH // 2)
                    nc.vector.tensor_copy(U[:, q0:q0 + QW], avP[:])
                    # per-quarter 1/Z chain + att scale (overlaps next quarter)
                    nc.sync.dma_start(z_row[0:1, q0:q0 + QW],
                                      U[48:49, q0:q0 + QW])
                    zTq = psS.tile([128, 8], F32, tag="S")
                    for jj in range(8):
                        j = 8 * qq + jj
                        nc.tensor.transpose(zTq[:, jj:jj + 1],
                                            z_row[0:1, 128 * j:128 * (j + 1)],
                                            ident[0:1, 0:1])
                    nc.vector.reciprocal(rz[:, 8 * qq:8 * qq + 8], zTq[:])
                    for g2 in range(2):
                        rzP = psS.tile([1, 512], F32, tag="S")
                        for jj in range(4):
                            j = 8 * qq + 4 * g2 + jj
                            nc.tensor.transpose(
                                rzP[0:1, 128 * jj:128 * (jj + 1)],
                                rz[:, j:j + 1], ident[:])
                        nc.scalar.copy(
                            rz_row[0:1, q0 + 512 * g2:q0 + 512 * (g2 + 1)],
                            rzP[:])
                        rbP = psS.tile([48, 512], F32, tag="S")
                        sl = slice(q0 + 512 * g2, q0 + 512 * (g2 + 1))
                        nc.tensor.matmul(rbP[:], ones_row[0:1, 0:48],
                                         rz_row[0:1, sl], start=True, stop=True)
                        nc.vector.tensor_mul(att[0:48, sl], U[0:48, sl], rbP[:])

            # ---- phase 8: 1/Z scale + proj
            with (
                tc.tile_pool(name="ph8", bufs=1) as p8,
                tc.tile_pool(name="psE", bufs=4, space="PSUM") as psE,
                tc.tile_pool(name="psF", bufs=2, space="PSUM") as psF,
            ):
                out_sb = p8.tile([128, HW], F32, tag="out_sb")
                out_sb2 = p8.tile([64, HW], F32, tag="out_sb2")

                nc.sync.dma_start(att[64:112, :], att[0:48, :])
                for n in range(8):
                    s = slice(512 * n, 512 * (n + 1))
                    oP = psE.tile([128, 512], F32, tag="e")
                    oP2 = psF.tile([64, 512], F32, tag="f")
                    nc.tensor.matmul(oP[:], pw[0:48, 0:128], att[0:48, s],
                                     start=True, stop=True)
                    nc.tensor.matmul(oP2[:], pw[64:112, 0:64], att[64:112, s],
                                     start=True, stop=True)
                    if n % 2 == 0:
                        nc.vector.tensor_copy(out_sb[:, s], oP[:])
                        nc.scalar.copy(out_sb2[:, s], oP2[:])
                    else:
                        nc.scalar.copy(out_sb[:, s], oP[:])
                        nc.vector.tensor_copy(out_sb2[:, s], oP2[:])
                nc.sync.dma_start(out_d[0:128, :], out_sb[:])
                nc.sync.dma_start(out_d[128:192, :], out_sb2[:])

    nc.compile()
    return nc


def _get_nc():
    if "nc" not in _cache:
        _cache["nc"] = _build()
    return _cache["nc"]


def _prep_core(x, qkv_w, dw_w, proj_w, temperature, b, h):
    w1 = qkv_w[:, :, 0, 0]  # [576, 192]
    dw = dw_w[:, 0]  # [576, 3, 3]
    pwf = proj_w[:, :, 0, 0]  # [192, 192]
    qs, ks, vs = h * C, DIM + h * C, 2 * DIM + h * C
    sel = np.concatenate(
        [w1[qs:qs + C], w1[ks:ks + C], w1[vs:vs + C]], 0)  # [144, 192]
    lhsT = np.ascontiguousarray(sel.T)  # [192, 144]
    dq, dk, dv = dw[qs:qs + C], dw[ks:ks + C], dw[vs:vs + C]
    dwqk = np.zeros((128, 9, C), np.float32)
    dwv = np.zeros((C, 9, C), np.float32)
    ar = np.arange(C)
    for t in range(9):
        dy, dx = t // 3 - 1, t % 3 - 1
        dwqk[ar, t, ar] = dq[:, dy + 1, dx + 1]
        dwqk[64 + ar, t, ar] = dk[:, dy + 1, dx + 1]
        dwv[ar, t, ar] = dv[:, dy + 1, dx + 1]
    pw_sel = pwf[:, h * C:(h + 1) * C].T  # [48, 192]
    pwt = np.zeros((128, 128), np.float32)
    pwt[0:48, 0:128] = pw_sel[:, 0:128]
    pwt[64:112, 0:64] = pw_sel[:, 128:192]
    return {
        "x": np.ascontiguousarray(x[b].reshape(DIM, HW)).astype(np.float16),
        "w1a": lhsT[0:128].astype(np.float16),
        "w1b": lhsT[128:192].astype(np.float16),
        "dwqk": dwqk.reshape(128, 9 * C).astype(np.float16),
        "dwv": dwv.reshape(C, 9 * C).astype(np.float16),
        "pw": pwt.astype(np.float16),
        "ident": np.eye(128, dtype=np.float32),
        "temp": np.array([[temperature[h, 0, 0]]], np.float32),
    }


def kernel(x, qkv_w, dw_w, proj_w, temperature):
    from concourse.bass_utils import run_bass_kernel_spmd

    nc = _get_nc()
    x = np.asarray(x, np.float32)
    qkv_w = np.asarray(qkv_w, np.float32)
    dw_w = np.asarray(dw_w, np.float32)
    proj_w = np.asarray(proj_w, np.float32)
    temperature = np.asarray(temperature, np.float32)
    in_maps = [
        _prep_core(x, qkv_w, dw_w, proj_w, temperature, c // HEADS, c % HEADS)
        for c in range(NCORES)
    ]
    res = run_bass_kernel_spmd(nc, in_maps, core_ids=list(range(NCORES)))
    out = np.zeros((B, DIM, HW), np.float32)
    for c in range(NCORES):
        out[c // HEADS] += res.results[c]["out"]
    return out.reshape(B, DIM, H, W)
